# revision 1
# baseline (speedup 1.0000x reference)
"""Segment-sharded multi-head attention (GNN message passing) for 8 trn2 cores.

Problem: B=64 global queries, E=65536 edges, N2=256, H=8 heads.
reference returns (x [B,N2], attn [H,B,E]); attn is ~98.5% structural zeros
(each edge only attends within its own segment b=batch[e]).

Sharding strategy (per the "shard by segment" hint): sort edges by batch id
on the host (that IS the sharding step), give each core 8 consecutive
segments (~8192 edges). Every softmax row (h, b) is then fully local to one
core -> no cross-device reduction at all, and each core computes only the
nonzero [64 x ~8192] block of attn.

Device kernel per core (CAP = padded edge count):
  scoresT[e, hb] = sum_d key[e,d] * A[hb,d]      (A = q @ Wk_head, host-folded)
  explocal = exp(scoresT) * segmask              (softmax max-shift cancels;
                                                  scores are O(30) so exp is
                                                  safe in f32 without it)
  U_unnorm[hb, 0:256] += explocal.T @ value      (PE accumulation)
  U_unnorm[hb, 256]   += sum_e explocal          (ones column of value)
Host epilogue: attn = explocal / gsum scattered to original edge order,
x = rowwise (U/gsum) @ Wv_h.T (+bv) concat-heads @ Wo.T + bo.
"""

import numpy as np

import concourse.bacc as bacc
import concourse.mybir as mybir
import concourse.tile as tile
from concourse.bass_utils import run_bass_kernel_spmd

B = 64          # segments / queries
E = 65536       # edges
H = 8           # heads
DK = 32         # head dim
N2 = 256        # model dim
NC = 8          # cores
SPC = 8         # segments per core
HB = 64         # rows per core = H * SPC

F32 = mybir.dt.float32

_kernel_cache: dict[int, object] = {}


def _build_kernel(cap: int):
    """Bass kernel for one core; cap must be a multiple of 512."""
    assert cap % 512 == 0
    ST = cap // 512  # super-tiles of 512 edges

    nc = bacc.Bacc("TRN2", target_bir_lowering=False, debug=False,
                   num_devices=NC)

    kt = nc.dram_tensor("kt", [N2, cap], F32, kind="ExternalInput")
    val = nc.dram_tensor("val", [cap, N2], F32, kind="ExternalInput")
    msk = nc.dram_tensor("msk", [cap, HB], F32, kind="ExternalInput")
    at = nc.dram_tensor("at", [N2, HB], F32, kind="ExternalInput")
    attn_un = nc.dram_tensor("attn_un", [cap, HB], F32, kind="ExternalOutput")
    u_out = nc.dram_tensor("u_out", [HB, N2 + 1], F32, kind="ExternalOutput")

    with tile.TileContext(nc) as tc:
        with (
            tc.tile_pool(name="const", bufs=1) as constp,
            tc.tile_pool(name="ktp", bufs=3) as ktp,
            tc.tile_pool(name="valp", bufs=3) as valp,
            tc.tile_pool(name="mskp", bufs=3) as mskp,
            tc.tile_pool(name="expp", bufs=3) as expp,
            tc.tile_pool(name="psp", bufs=3, space="PSUM") as psp,
            tc.tile_pool(name="psu", bufs=1, space="PSUM") as psup,
        ):
            # A^T, split into the two 128-row contraction chunks.
            at_t = constp.tile([128, 2, HB], F32)
            nc.sync.dma_start(
                at_t[:], at[:, :].rearrange("(c p) h -> p c h", p=128))

            u_ps = psup.tile([HB, N2 + 1], F32)

            for s in range(ST):
                sl = slice(s * 512, (s + 1) * 512)

                kt_t = ktp.tile([128, 2, 512], F32)
                nc.sync.dma_start(
                    kt_t[:], kt[:, sl].rearrange("(c p) e -> p c e", p=128))

                val_t = valp.tile([128, 4, N2 + 1], F32)
                nc.sync.dma_start(
                    val_t[:, :, 0:N2],
                    val[sl, :].rearrange("(t p) d -> p t d", p=128))
                nc.vector.memset(val_t[:, :, N2], 1.0)

                msk_t = mskp.tile([128, 4, HB], F32)
                nc.sync.dma_start(
                    msk_t[:], msk[sl, :].rearrange("(t p) h -> p t h", p=128))

                # scoresT for 4 sub-tiles of 128 edges, packed in free dim.
                ps_t = psp.tile([128, 4, HB], F32)
                for t in range(4):
                    esl = slice(t * 128, (t + 1) * 128)
                    nc.tensor.matmul(ps_t[:, t, :], kt_t[:, 0, esl],
                                     at_t[:, 0, :], start=True, stop=False,
                                     skip_group_check=True)
                    nc.tensor.matmul(ps_t[:, t, :], kt_t[:, 1, esl],
                                     at_t[:, 1, :], start=False, stop=True,
                                     skip_group_check=True)

                exp_t = expp.tile([128, 4, HB], F32)
                nc.scalar.activation(exp_t[:], ps_t[:],
                                     mybir.ActivationFunctionType.Exp)
                nc.vector.tensor_mul(exp_t[:], exp_t[:], msk_t[:])

                for t in range(4):
                    nc.tensor.matmul(u_ps[:], exp_t[:, t, :], val_t[:, t, :],
                                     start=(s == 0 and t == 0),
                                     stop=(s == ST - 1 and t == 3),
                                     skip_group_check=True)

                nc.sync.dma_start(
                    attn_un[sl, :].rearrange("(t p) h -> p t h", p=128),
                    exp_t[:])

            u_sb = constp.tile([HB, N2 + 1], F32)
            nc.vector.tensor_copy(u_sb[:], u_ps[:])
            nc.sync.dma_start(u_out[:], u_sb[:])

    nc.compile()
    return nc


def _get_kernel(cap: int):
    if cap not in _kernel_cache:
        _kernel_cache[cap] = _build_kernel(cap)
    return _kernel_cache[cap]


def _prep(global_query, local_key, local_value, batch,
          Wq, bq, Wk, bk, Wv, bv, Wo, bo):
    order = np.argsort(batch, kind="stable")
    sb = batch[order]
    counts = np.bincount(batch, minlength=B)
    lo = np.zeros(B + 1, np.int64)
    lo[1:] = np.cumsum(counts)
    core_lo = lo[::SPC]                       # [NC+1]
    cnts = np.diff(core_lo)
    cap = int(np.ceil(max(int(cnts.max()), 512) / 512.0) * 512)

    q = global_query.astype(np.float64) @ Wq.T.astype(np.float64) + bq
    q = q.astype(np.float32)                  # [B, N2]
    # A[h, b, d] = q_h[b] . Wk_h[:, d]   (bk dropped: per-row shift cancels
    # in softmax)
    A = np.einsum("bhk,hkd->hbd", q.reshape(B, H, DK).astype(np.float64),
                  Wk.reshape(H, DK, N2).astype(np.float64)).astype(np.float32)

    keyT = np.ascontiguousarray(local_key[order].T)   # [N2, E]
    vals = local_value[order]                         # [E, N2]

    in_maps = []
    for c in range(NC):
        s, e = int(core_lo[c]), int(core_lo[c + 1])
        n = e - s
        ktc = np.zeros((N2, cap), np.float32)
        ktc[:, :n] = keyT[:, s:e]
        vc = np.zeros((cap, N2), np.float32)
        vc[:n] = vals[s:e]
        segs = np.arange(c * SPC, (c + 1) * SPC, dtype=batch.dtype)
        m8 = (sb[s:e, None] == segs[None, :]).astype(np.float32)
        mc = np.zeros((cap, HB), np.float32)
        mc[:n] = np.tile(m8, (1, H))          # col h*SPC+j
        Ac = A[:, segs, :]                    # [H, SPC, N2]
        atc = np.ascontiguousarray(Ac.reshape(HB, N2).T).astype(np.float32)
        in_maps.append({"kt": ktc, "val": vc, "msk": mc, "at": atc})

    return in_maps, order, lo, core_lo, cap


def kernel(**inputs):
    gq = np.asarray(inputs["global_query"], np.float32)
    key = np.asarray(inputs["local_key"], np.float32)
    value = np.asarray(inputs["local_value"], np.float32)
    batch = np.asarray(inputs["batch"])
    Wq = np.asarray(inputs["Wq"], np.float32)
    bq = np.asarray(inputs["bq"], np.float32)
    Wk = np.asarray(inputs["Wk"], np.float32)
    bk = np.asarray(inputs["bk"], np.float32)
    Wv = np.asarray(inputs["Wv"], np.float32)
    bv = np.asarray(inputs["bv"], np.float32)
    Wo = np.asarray(inputs["Wo"], np.float32)
    bo = np.asarray(inputs["bo"], np.float32)

    in_maps, order, lo, core_lo, cap = _prep(
        gq, key, value, batch, Wq, bq, Wk, bk, Wv, bv, Wo, bo)

    nc = _get_kernel(cap)
    res = run_bass_kernel_spmd(nc, in_maps, list(range(NC))).results

    attn = np.zeros((H, B, E), np.float32)
    xcat = np.zeros((B, N2), np.float32)
    WvH = Wv.reshape(H, DK, N2)
    bvH = bv.reshape(H, DK)

    for c in range(NC):
        au = res[c]["attn_un"]                # [cap, HB]
        u = res[c]["u_out"]                   # [HB, N2+1]
        gsum = u[:, N2]                       # [HB]
        nz = (gsum > 0).astype(np.float32)
        gsafe = np.where(gsum > 0, gsum, 1.0)
        Un = u[:, :N2] / gsafe[:, None]       # [HB, N2]

        s0 = int(core_lo[c])
        for j in range(SPC):
            b = c * SPC + j
            gs, ge = int(lo[b]), int(lo[b + 1])
            if ge == gs:
                continue
            eidx = order[gs:ge]
            rows = slice(gs - s0, ge - s0)
            valsb = au[rows, j::SPC]          # [cnt_b, H]
            inv = 1.0 / gsafe[j::SPC]         # [H]
            attn[:, b, eidx] = (valsb * inv[None, :]).T

        # out[h, j, dk] = Un[h*SPC+j] @ Wv_h.T + bv_h * (gsum>0)
        UnH = Un.reshape(H, SPC, N2)
        nzH = nz.reshape(H, SPC)
        o = np.einsum("hjd,hkd->hjk", UnH, WvH) + \
            bvH[:, None, :] * nzH[:, :, None]
        xcat[c * SPC:(c + 1) * SPC] = o.transpose(1, 0, 2).reshape(SPC, N2)

    x = (xcat @ Wo.T + bo).astype(np.float32)
    return x, attn
